# revision 5
# baseline (speedup 1.0000x reference)
"""Bidirectional complex-diagonal LRU (Linear Recurrent Unit) on 8 Trainium2 cores.

Math: lam = exp(-exp(nu_log) + i*exp(theta_log)) per channel n (N=512).
  Bu = einsum('blh,hn->bnl', u, B0 + iB1), masked to length.
  Forward scan over channels [0,256), backward (time-reversed) over [256,512).
  y = x.real @ C0 - x.imag @ C1, zeroed past each sequence length.

Device strategy (data-parallel, one batch per core):
  - Rotation trick: x_t = e^{i*th*t} * w_t turns the complex recurrence
    x_t = lam x_{t-1} + Bu_t into TWO real recurrences w_t = r w_{t-1} + v_t
    (r = |lam|), each a native DVE tensor_tensor_scan along the free dim.
  - Twiddle tables cos/sin(th*j) built on host in fp64 (exact phases), fp16 on
    device. Per-core masking (zero columns past the sequence length) is folded
    into the tables, so masking costs nothing on device.
  - Backward channels run on the reversed time axis; reversal happens for free
    inside the PSUM-evacuation copy (negative-stride AP) and again in the
    untwiddle output write.
  - All matmuls in fp16 (PE 4x rate vs fp32), accumulation in fp32 PSUM.

Self-contained: hardcodes B=8, L=4096, H=N=512, 8 cores.
"""

import numpy as np
from contextlib import ExitStack

import concourse.bass as bass
import concourse.bacc as bacc
import concourse.mybir as mybir
import concourse.tile as tile

P = 128
L = 4096
H = 512
N = 512
BSZ = 8
SEG = 512
NSEG = L // SEG          # 8
KH = H // P              # 4 contraction chunks for Bu
NCH = 2 * N // P         # 8 real-channel chunks (re 0..3, im 4..7)
CCH = N // P             # 4 complex-channel chunks (0,1 fwd; 2,3 bwd)
NT = L // P              # 32 time blocks for the output matmul

F16 = mybir.dt.float16
F32 = mybir.dt.float32

TSEG_ORDER = [0, 7, 1, 6, 2, 5, 3, 4]
NCH_ORDER = [0, 4, 2, 6, 1, 5, 3, 7]   # pair re/im chunks so scans unblock early
C_ORDER = [0, 2, 1, 3]

_CACHED = None


def _is_fwd_chunk(nch: int) -> bool:
    # real chunks 0,1 (re of fwd channels) and 4,5 (im of fwd) are forward
    return (nch % 4) < 2


def build_nc():
    nc = bacc.Bacc("TRN2", target_bir_lowering=False, debug=False)
    uT = nc.declare_dram_parameter("uT", [H, L], F16, isOutput=False)
    cosT = nc.declare_dram_parameter("cosT", [N, L], F16, isOutput=False)
    sinT = nc.declare_dram_parameter("sinT", [N, L], F16, isOutput=False)
    rdec = nc.declare_dram_parameter("rdec", [P, CCH], F32, isOutput=False)
    Bcat = nc.declare_dram_parameter("Bcat", [H, 2 * N], F16, isOutput=False)
    Ccat = nc.declare_dram_parameter("Ccat", [2 * N, H], F16, isOutput=False)
    y = nc.declare_dram_parameter("y", [L, H], F32, isOutput=True)

    with tile.TileContext(nc) as tc, ExitStack() as ctx:
        const = ctx.enter_context(tc.tile_pool(name="const", bufs=1))
        big = ctx.enter_context(tc.tile_pool(name="big", bufs=1))
        pscr = ctx.enter_context(tc.tile_pool(name="pscr", bufs=8))
        wpool = ctx.enter_context(tc.tile_pool(name="wpool", bufs=12))
        ysb = ctx.enter_context(tc.tile_pool(name="ysb", bufs=3))
        bup = ctx.enter_context(tc.tile_pool(name="bup", bufs=4, space="PSUM"))
        yp = ctx.enter_context(tc.tile_pool(name="yp", bufs=4, space="PSUM"))

        u_t = [big.tile([P, L], F16, tag=f"uT{k}", name=f"uT{k}") for k in range(KH)]
        cosb = [big.tile([P, L], F16, tag=f"cos{c}", name=f"cos{c}") for c in range(CCH)]
        sinb = [big.tile([P, L], F16, tag=f"sin{c}", name=f"sin{c}") for c in range(CCH)]
        v = [big.tile([P, L], F16, tag=f"v{j}", name=f"v{j}") for j in range(NCH)]
        bmat = [const.tile([P, 2 * N], F16, tag=f"B{k}", name=f"Bm{k}") for k in range(KH)]
        cmat = [const.tile([P, H], F16, tag=f"C{k}", name=f"Cm{k}") for k in range(NCH)]
        rdec_t = const.tile([P, CCH], F32, tag="rdec", name="rdec_t")

        # ---- input DMAs ----
        nc.sync.dma_start(rdec_t[:], rdec[:])
        for k in range(KH):
            nc.sync.dma_start(bmat[k][:], Bcat[k * P:(k + 1) * P, :])
        for k in range(NCH):
            nc.sync.dma_start(cmat[k][:], Ccat[k * P:(k + 1) * P, :])
        for k in range(KH):
            nc.sync.dma_start(u_t[k][:], uT[k * P:(k + 1) * P, :])
        for c in range(CCH):
            nc.sync.dma_start(cosb[c][:], cosT[c * P:(c + 1) * P, :])
            nc.sync.dma_start(sinb[c][:], sinT[c * P:(c + 1) * P, :])

        # ---- Phase A: Bu matmuls, evacuate into v slots (scan-time order) ----
        for nch in NCH_ORDER:
            for half in range(2):
                tsegs = TSEG_ORDER[half * 4:(half + 1) * 4]
                psums = {}
                for k in range(KH):
                    for ts in tsegs:
                        if k == 0:
                            psums[ts] = bup.tile([P, SEG], F32, name=f"bups{ts}", tag="bup")
                        nc.tensor.matmul(
                            psums[ts][:],
                            bmat[k][:, nch * P:(nch + 1) * P],
                            u_t[k][:, ts * SEG:(ts + 1) * SEG],
                            start=(k == 0), stop=(k == KH - 1),
                        )
                for ts in tsegs:
                    if _is_fwd_chunk(nch):
                        nc.scalar.copy(v[nch][:, ts * SEG:(ts + 1) * SEG],
                                       psums[ts][:])
                    else:
                        ss = NSEG - 1 - ts
                        dst = v[nch][:, ss * SEG:(ss + 1) * SEG]
                        nc.scalar.copy(dst[:, ::-1], psums[ts][:])

        # ---- Phases B/C/D per (scan-seg, complex chunk) ----
        prev_wr = [None] * CCH
        prev_wi = [None] * CCH
        for ss in range(NSEG):
            for c in C_ORDER:
                jre, jim = c, c + CCH
                sl = slice(ss * SEG, (ss + 1) * SEG)
                cs = cosb[c][:, sl]
                sn = sinb[c][:, sl]
                vre = v[jre][:, sl]
                vim = v[jim][:, sl]

                # twiddle-in: vr = c*br + s*bi ; vi = c*bi - s*br (in place)
                p1 = pscr.tile([P, SEG], F16, tag="p", name="p1")
                p2 = pscr.tile([P, SEG], F16, tag="p", name="p2")
                p3 = pscr.tile([P, SEG], F16, tag="p", name="p3")
                p4 = pscr.tile([P, SEG], F16, tag="p", name="p4")
                nc.vector.tensor_mul(p1[:], cs, vre)
                nc.vector.tensor_mul(p2[:], sn, vim)
                nc.vector.tensor_mul(p3[:], cs, vim)
                nc.vector.tensor_mul(p4[:], sn, vre)
                nc.vector.tensor_add(vre, p1[:], p2[:])
                nc.vector.tensor_sub(vim, p3[:], p4[:])

                # scans: w = scan(r, v)
                r_ap = rdec_t[:, c:c + 1].broadcast_to((P, SEG))
                wr = wpool.tile([P, SEG], F16, tag="w", name="wr")
                wi = wpool.tile([P, SEG], F16, tag="w", name="wi")
                init_r = 0.0 if ss == 0 else prev_wr[c][:, SEG - 1:SEG]
                init_i = 0.0 if ss == 0 else prev_wi[c][:, SEG - 1:SEG]
                nc.vector.tensor_tensor_scan(
                    wr[:], r_ap, vre, init_r,
                    op0=mybir.AluOpType.mult, op1=mybir.AluOpType.add)
                nc.vector.tensor_tensor_scan(
                    wi[:], r_ap, vim, init_i,
                    op0=mybir.AluOpType.mult, op1=mybir.AluOpType.add)
                prev_wr[c], prev_wi[c] = wr, wi

                # untwiddle: xr = c*wr - s*wi ; xi = s*wr + c*wi
                q1 = pscr.tile([P, SEG], F16, tag="p", name="q1")
                q2 = pscr.tile([P, SEG], F16, tag="p", name="q2")
                q3 = pscr.tile([P, SEG], F16, tag="p", name="q3")
                q4 = pscr.tile([P, SEG], F16, tag="p", name="q4")
                nc.vector.tensor_mul(q1[:], cs, wr[:])
                nc.vector.tensor_mul(q2[:], sn, wi[:])
                nc.vector.tensor_mul(q3[:], sn, wr[:])
                nc.vector.tensor_mul(q4[:], cs, wi[:])
                if c < 2:   # forward: t-order, overwrite the spent v slices
                    nc.vector.tensor_sub(vre, q1[:], q2[:])
                    nc.vector.tensor_add(vim, q3[:], q4[:])
                else:       # backward: write t-ordered (reversed) into the spent
                    # cos/sin table slices; t-seg (NSEG-1-ss) lives at slice ss
                    nc.vector.tensor_sub(cs[:, ::-1], q1[:], q2[:])
                    nc.vector.tensor_add(sn[:, ::-1], q3[:], q4[:])

        # x source for the output matmul: real chunk k, time block i
        def x_src(k: int, i: int):
            j, o = divmod(i, SEG // P)
            if _is_fwd_chunk(k):
                return v[k][:, i * P:(i + 1) * P]
            c = k % 4
            col = (NSEG - 1 - j) * SEG + o * P
            src = cosb[c] if k < 4 else sinb[c]
            return src[:, col:col + P]

        # ---- Phase E: y matmuls (middle-out order so work starts earliest) ----
        seg_order = [3, 4, 2, 5, 1, 6, 0, 7]
        for sj in seg_order:
            for o in range(SEG // P):
                i = sj * (SEG // P) + o
                py = yp.tile([P, H], F32, name="py", tag="yp")
                for k in range(NCH):
                    nc.tensor.matmul(
                        py[:], x_src(k, i), cmat[k][:],
                        start=(k == 0), stop=(k == NCH - 1),
                    )
                yt = ysb.tile([P, H], F32, tag="y", name="yt")
                nc.scalar.copy(yt[:], py[:])
                nc.sync.dma_start(y[i * P:(i + 1) * P, :], yt[:])

    nc.compile()
    return nc


def prepare_inputs(u, lengths, nu_log, theta_log, B, C):
    """Host-side prep: per-core in_maps. All heavy math in fp64 for accuracy."""
    u = np.asarray(u)
    lengths = np.asarray(lengths)
    nu = np.exp(np.asarray(nu_log, np.float64))
    theta = np.exp(np.asarray(theta_log, np.float64))
    r = np.exp(-nu)                                    # |lam|, (N,)

    # twiddle tables, phase exact in fp64 then reduced mod 2pi
    j = np.arange(L, dtype=np.float64)
    ang = np.mod(theta[:, None] * j[None, :], 2 * np.pi)   # (N, L)
    cos_base = np.cos(ang).astype(np.float16)
    sin_base = np.sin(ang).astype(np.float16)

    Bcat = np.empty((H, 2 * N), np.float16)
    Bcat[:, :N] = np.asarray(B)[..., 0]
    Bcat[:, N:] = np.asarray(B)[..., 1]
    Ccat = np.empty((2 * N, H), np.float16)
    Ccat[:N] = np.asarray(C)[0]
    Ccat[N:] = -np.asarray(C)[1]
    rdec = r.reshape(CCH, P).T.astype(np.float32).copy()   # (128, 4)

    half = N // 2
    in_maps = []
    for b in range(BSZ):
        ln = int(lengths[b])
        ub = np.array(u[b], np.float32)
        if ln < L:
            ub[ln:, :] = 0.0
        uT = np.ascontiguousarray(ub.T.astype(np.float16))
        cosb = cos_base.copy()
        sinb = sin_base.copy()
        if ln < L:
            # forward channels: zero scan-positions j >= len
            cosb[:half, ln:] = 0
            sinb[:half, ln:] = 0
            # backward channels: scan-position j maps to t = L-1-j;
            # zero where t >= len  <=>  j < L - len
            cosb[half:, :L - ln] = 0
            sinb[half:, :L - ln] = 0
        in_maps.append({
            "uT": uT, "cosT": cosb, "sinT": sinb,
            "rdec": rdec, "Bcat": Bcat, "Ccat": Ccat,
        })
    return in_maps


def kernel(u, lengths, nu_log, theta_log, B, C):
    global _CACHED
    from concourse.bass_utils import run_bass_kernel_spmd
    in_maps = prepare_inputs(u, lengths, nu_log, theta_log, B, C)
    if _CACHED is None:
        _CACHED = build_nc()
    res = run_bass_kernel_spmd(_CACHED, in_maps, list(range(BSZ)))
    y = np.stack([res.results[i]["y"] for i in range(BSZ)], axis=0)
    return y.astype(np.float32)
